# revision 13
# baseline (speedup 1.0000x reference)
"""KNN column-imputation kernel (nn_ColProcessor) for 8 Trainium2 cores.

Strategy: shard the 4096 query rows across 8 cores (512 rows each, data
parallel - rows are fully independent). Each core streams its [512, 16384]
slice of dist_chunk through SBUF in [128, 16384] tiles:

  - DMA: 4096-column chunks (16KB contiguous per partition line, all on
    the single SP hardware queue - measured 344 GB/s/core; splitting
    across the SP+Act queues or finer chunks measured SLOWER). Only the
    last tile tapers to (..., 2048, 1024, 1024) chunks so the
    after-last-byte drain is one small fold, not a 4096-wide one.
  - VectorE (DVE): per chunk, ONE windowed `tensor_reduce(op=min,
    negate=True)` folds [128, W/32, 32] -> W/32 negated group minima
    (W cycles; elementwise/reduce ops process 1 elem/cycle but read the
    whole group per output, vs the 1 elem/cycle full-width scans of the
    top-8 select ops, which have no fast modes). After all chunks the
    folded array fv [128, 512] holds -min(group) for 512 groups of 32
    consecutive columns. A single `max` (top-8, descending) + `max_index`
    (sequential duplicate-aware matching) over the 512-wide fv yields
    the 8 groups with the smallest minima. DVE per tile: 16384 fold +
    2*512 select cycles vs 32.8K cycles for two full-width passes.
  - Tiny i8/v8 out-DMAs ride the otherwise idle Act hardware queue.

Guarantee: every element with distance < T8 (the 8th-best group min) lies
in a returned group, so the 8 groups' 256 member columns contain the
donor-filtered top-5 whenever the 5th donor distance < T8. The host
gathers the exact f32 distances of the 256 candidates, applies the donor
mask, and reproduces the reference's stable (value, index) top-5 exactly.
Rows violating the guarantee (5th donor >= T8, ~7% from top-8 elements
sharing a group) and NaN inputs fall back to an exact numpy replay.

Engine budget per core: DMA 32MB at ~344 GB/s = 95us (the HBM floor for
this memory-bound problem), DVE ~72us hidden under it, Act idle. The
pipeline is DMA-bound: measured 95.4us vs 165.5us for the 2-pass
max/max_index version.
"""

import sys

sys.path.insert(0, "/opt/trn_rl_repo")

import numpy as np

import concourse.bacc as bacc
import concourse.mybir as mybir
from concourse.tile import TileContext

N_Q, N_FIT, N_FEAT = 4096, 16384, 32
COL, K = 3, 5
BIG = 1.0e30
NAN_FILL = 1.0e10
N_CORES = 8
ROWS = N_Q // N_CORES  # 512 query rows per core
P = 128
N_TILES = ROWS // P  # 4
GS = 32  # columns per fold group
NG = N_FIT // GS  # 512 groups per row
# DMA/fold chunking: bulk tiles use 16KB-per-partition transfers; the final
# tile of the steady-state loop tapers so the drain after the last byte is
# only a 1024-wide fold + the 512-wide top-8 selection.
CHUNKS_BULK = (4096, 4096, 4096, 4096)
CHUNKS_TAIL = (4096, 4096, 4096, 2048, 1024, 1024)
SENT = np.int64(0xFFFFFFFF)

_EXEC_CACHE = {}


def _build(reps=1, loop_n=None):
    """Build the per-core NEFF. loop_n wraps the body in an on-device For_i
    loop (used only for timing: the NEFF size is loop-bound independent, so
    wall-clock slopes between loop counts isolate pure HW execution time)."""
    import contextlib

    nc = bacc.Bacc("TRN2", target_bir_lowering=False)
    d_in = nc.dram_tensor("d", [ROWS, N_FIT], mybir.dt.float32, kind="ExternalInput")
    if loop_n:
        # timing-only builds take a per-call-unique salt so the axon relay's
        # identical-execution cache can't elide repeated timed runs
        salt_in = nc.dram_tensor("salt", [1, 8], mybir.dt.float32, kind="ExternalInput")
    i_out = nc.dram_tensor("idx", [ROWS, 8], mybir.dt.uint32, kind="ExternalOutput")
    v_out = nc.dram_tensor("vals", [ROWS, 8], mybir.dt.float32, kind="ExternalOutput")

    with TileContext(nc) as tc:
        with (
            tc.tile_pool(name="work", bufs=2) as work,
            tc.tile_pool(name="small", bufs=4) as small,
        ):
            if loop_n:
                salt_t = small.tile([1, 8], mybir.dt.float32)
                nc.sync.dma_start(out=salt_t, in_=salt_in[:, :])
            loop = tc.For_i(0, loop_n, 1) if loop_n else contextlib.nullcontext()
            with loop:
                for t in range(N_TILES * reps):
                    t = t % N_TILES
                    rs = slice(t * P, (t + 1) * P)
                    chunks = CHUNKS_TAIL if t == N_TILES - 1 else CHUNKS_BULK
                    dt = work.tile([P, N_FIT], mybir.dt.float32)
                    fv = small.tile([P, NG], mybir.dt.float32)
                    col = 0
                    for w in chunks:
                        cs = slice(col, col + w)
                        nc.sync.dma_start(out=dt[:, cs], in_=d_in[rs, cs])
                        nc.vector.tensor_reduce(
                            out=fv[:, col // GS : (col + w) // GS],
                            in_=dt[:, cs].rearrange("p (g m) -> p g m", m=GS),
                            axis=mybir.AxisListType.X,
                            op=mybir.AluOpType.min,
                            negate=True,
                        )
                        col += w
                    v8 = small.tile([P, 8], mybir.dt.float32)
                    i8 = small.tile([P, 8], mybir.dt.uint32)
                    nc.vector.max(out=v8, in_=fv)
                    nc.vector.max_index(out=i8, in_max=v8, in_values=fv)
                    nc.scalar.dma_start(out=i_out[rs, :], in_=i8)
                    nc.scalar.dma_start(out=v_out[rs, :], in_=v8)
    nc.finalize()
    return nc


def _get_exec(nc):
    """Cached jitted 8-core executor for a finalized Bass module.

    Mirrors bass2jax.run_bass_via_pjrt's multi-core path but memoizes the
    jitted function so repeated calls don't re-trace/re-compile, and accepts
    already-device-resident concat inputs.
    """
    key = id(nc)
    if key in _EXEC_CACHE:
        return _EXEC_CACHE[key]

    import jax
    from jax.sharding import Mesh, PartitionSpec
    from jax.experimental.shard_map import shard_map
    from concourse import bass2jax
    from concourse import mybir as _mybir

    bass2jax.install_neuronx_cc_hook()

    partition_name = nc.partition_id_tensor.name if nc.partition_id_tensor else None
    in_names, out_names, out_avals, zero_outs = [], [], [], []
    for alloc in nc.m.functions[0].allocations:
        if not isinstance(alloc, _mybir.MemoryLocationSet):
            continue
        name = alloc.memorylocations[0].name
        if alloc.kind == "ExternalInput":
            if name != partition_name:
                in_names.append(name)
        elif alloc.kind == "ExternalOutput":
            out_names.append(name)
            shape = tuple(alloc.tensor_shape)
            dtype = _mybir.dt.np(alloc.dtype)
            out_avals.append(jax.core.ShapedArray(shape, dtype))
            zero_outs.append(np.zeros(shape, dtype))
    n_params = len(in_names)
    n_outs = len(out_avals)
    all_in_names = list(in_names) + list(out_names)
    if partition_name is not None:
        all_in_names.append(partition_name)
    donate = tuple(range(n_params, n_params + n_outs))

    def _body(*args):
        operands = list(args)
        if partition_name is not None:
            operands.append(bass2jax.partition_id_tensor())
        outs = bass2jax._bass_exec_p.bind(
            *operands,
            out_avals=tuple(out_avals),
            in_names=tuple(all_in_names),
            out_names=tuple(out_names),
            lowering_input_output_aliases=(),
            sim_require_finite=True,
            sim_require_nnan=True,
            nc=nc,
        )
        return tuple(outs)

    devices = jax.devices()[:N_CORES]
    mesh = Mesh(np.asarray(devices), ("core",))
    in_specs = (PartitionSpec("core"),) * (n_params + n_outs)
    out_specs = (PartitionSpec("core"),) * n_outs
    jitted = jax.jit(
        shard_map(
            _body, mesh=mesh, in_specs=in_specs, out_specs=out_specs, check_rep=False
        ),
        donate_argnums=donate,
        keep_unused=True,
    )

    def run(concat_inputs):
        """concat_inputs: dict name -> (N_CORES*per_core_rows, ...) array."""
        args = [concat_inputs[n] for n in in_names]
        zeros = [
            np.zeros((N_CORES * z.shape[0], *z.shape[1:]), z.dtype) for z in zero_outs
        ]
        outs = jitted(*args, *zeros)
        return {n: outs[i] for i, n in enumerate(out_names)}

    _EXEC_CACHE[key] = run
    return run


_NC = None


def _device_top8_groups(d):
    """d: [N_Q, N_FIT] f32 -> (i8 [N_Q, 8] int64 group ids, T8 [N_Q] f32).

    i8 holds the 8 groups (of GS consecutive columns) with the smallest
    minima, T8 the 8th-best group minimum. Every element with distance
    strictly below T8 is inside a returned group. max_index runs over the
    full folded array, so duplicate folded values resolve to distinct
    groups (sequential matching) with no host-side ambiguity.
    """
    global _NC
    if _NC is None:
        _NC = _build()
    run = _get_exec(_NC)
    out = run({"d": np.ascontiguousarray(d)})
    i8 = np.asarray(out["idx"]).astype(np.int64)  # [N_Q, 8] group ids
    v8 = np.asarray(out["vals"])  # [N_Q, 8] f32, descending -min
    return i8, -v8[:, 7]


def _exact_rows(d_rows, donor_ok, mask_fit_col, fitcol):
    """Exact numpy replay of the reference for a few rows: returns val[n]."""
    dm = np.where(
        donor_ok[None, :],
        np.where(np.isnan(d_rows), np.float32(NAN_FILL), d_rows),
        np.float32(BIG),
    )
    all_nan = np.all(np.isnan(d_rows) | ~donor_ok[None, :], axis=1)
    order = np.argsort(dm, axis=1, kind="stable")[:, :K]
    w = 1.0 - mask_fit_col[order].astype(np.float32)
    donors = fitcol[order]
    wsum = w.sum(axis=1)
    div = np.where(wsum == 0, np.float32(1.0), wsum)
    knn_val = (donors * w).sum(axis=1) / div
    obs = ~mask_fit_col
    msum = obs.sum(dtype=np.float32)
    col_sum = np.where(obs, fitcol, 0.0).sum(dtype=np.float32)
    col_mean = col_sum / (msum if msum > 0 else np.float32(1.0))
    return np.where(all_nan, col_mean, knn_val).astype(np.float32)


def kernel(
    X,
    dist_chunk,
    non_missing_fix_X,
    mask_fit_X,
    dist_idx_map,
    mask,
    row_missing_idx,
    _fit_X,
):
    X = np.asarray(X, dtype=np.float32)
    dist_chunk = np.asarray(dist_chunk, dtype=np.float32)
    non_missing_fix_X = np.asarray(non_missing_fix_X, dtype=bool)
    mask_fit_X = np.asarray(mask_fit_X, dtype=bool)
    mask = np.asarray(mask, dtype=bool)
    _fit_X = np.asarray(_fit_X, dtype=np.float32)
    rmi = np.asarray(row_missing_idx, dtype=np.int64)
    dmap = np.asarray(dist_idx_map, dtype=np.int64)

    gather_rows = dmap[rmi]
    if gather_rows.shape[0] == N_Q and np.array_equal(
        gather_rows, np.arange(N_Q, dtype=np.int64)
    ):
        d = dist_chunk
    else:
        d = np.ascontiguousarray(dist_chunk[gather_rows])
    assert d.shape == (N_Q, N_FIT)

    donor_ok = non_missing_fix_X[:, COL]
    fitcol = _fit_X[:, COL]
    mask_fit_col = mask_fit_X[:, COL]

    if np.isnan(d.sum(dtype=np.float64)):
        # NaNs would poison the on-device min fold; replay everything exactly
        val = _exact_rows(d, donor_ok, mask_fit_col, fitcol)
    else:
        i8, T8 = _device_top8_groups(d)
        i8 = np.where((i8 < 0) | (i8 >= NG), 0, i8)

        # expand the 8 winning groups to their member columns, sorted by
        # index so the stable argsort below reproduces top_k's tie-break
        cand = (
            i8[:, :, None] * GS + np.arange(GS, dtype=np.int64)[None, None, :]
        ).reshape(N_Q, 8 * GS)
        cand = np.sort(cand, axis=1)
        dval = np.take_along_axis(d, cand, axis=1)
        dm = np.where(donor_ok[cand], dval, np.float32(BIG))
        order = np.argsort(dm, axis=1, kind="stable")[:, :K]
        idx5 = np.take_along_axis(cand, order, axis=1)
        v5 = np.take_along_axis(dm, order, axis=1)
        # exactness guarantee: the 5 donors must all beat the 8th group min
        bad_rows = ~(v5[:, K - 1] < T8)

        w = 1.0 - mask_fit_col[idx5].astype(np.float32)
        donors = fitcol[idx5]
        wsum = w.sum(axis=1)
        div = np.where(wsum == 0, np.float32(1.0), wsum)
        val = ((donors * w).sum(axis=1) / div).astype(np.float32)

        if bad_rows.any():
            bad = np.flatnonzero(bad_rows)
            val[bad] = _exact_rows(d[bad], donor_ok, mask_fit_col, fitcol)

    col_mask = mask[rmi, COL]
    new_col = np.where(col_mask, val, X[rmi, COL]).astype(np.float32)
    out = X.copy()
    out[rmi, COL] = new_col
    return out
